# revision 5
# baseline (speedup 1.0000x reference)
"""LogSparse MultiHeadAttention v2 — fp8-centered attention, all-fp8 matmuls.

Per-core math (batch-parallel over 8 cores, head loop on device):
  scores^T = x B_h x^T / 4096 via t-block (tT = B^T xT, fp8 DR)
  es = exp(ps/4096 + arg_k)            Act, arg folds rank-1 k-term + ln(s_sc)
  att_s = es * expbias^T               DVE tt (bf16)
  att8 = att_s - s_sc                  fp8 centered: att8 = s_sc*(E-1)
  den: ones^T att8 (fp8 DR) -> [1,512] PSUM -> DMA-transpose -> [128,4]
       rcol = 1/((dps + s_sc*S) * s_u * s_c)
  ctx8 = s_c * (xN8^T att8)            fp8 DR matmul, Act scale-copy evict
  y[q,:] += (ctx8^T u8 + w_s) * rcol   fp8 DR + rank-1 w matmul, stt evict
w_s = s_sc*s_u*U_h^T colsum(x) re-adds the centering mean term exactly.
"""
import numpy as np
from contextlib import ExitStack

import concourse.bass as bass
import concourse.mybir as mybir
import concourse.tile as tile
from concourse import bacc

dt = mybir.dt
AF = mybir.ActivationFunctionType
ALU = mybir.AluOpType

S = 1024
D = 512
H = 8
SQ = float(1.0 / np.sqrt(D))
P = 128
NT = S // D   # 2 token tiles of 512
KC = D // P   # 4 contraction chunks per 512
TC = S // P   # 8 token chunks of 128

S_SC = 16.0   # att8 = s_sc*(E-1)
S_U = 64.0    # u8 = U * s_u
S_C = 0.25    # ctx8 = s_c * ps_ctx
KS = 4096.0   # b_all prescale


def build(n_iters: int = 1, md_name: str = "fp8v2", sc_bufs: int = 3, cx_bufs: int = 2,
          pe_bufs: int = 2, att_bufs: int = 2, es_bufs: int = 4, wh_bufs: int = 2,
          quant_pool_qt: int = 1, dbg: bool = False) -> bacc.Bacc:
    FP8 = dt.float8e4
    DR = mybir.MatmulPerfMode.DoubleRow
    nc = bacc.Bacc("TRN2", target_bir_lowering=False, debug=False)

    xT = nc.dram_tensor("xT", [D, S], FP8, kind="ExternalInput")
    xN = nc.dram_tensor("xN", [S, D], FP8, kind="ExternalInput")
    b_all = nc.dram_tensor("b_all", [D, H * D], FP8, kind="ExternalInput")
    u8_d = nc.dram_tensor("u8", [D, H * D], FP8, kind="ExternalInput")
    eb_d = nc.dram_tensor("eb", [S, S], dt.bfloat16, kind="ExternalInput")
    arg_d = nc.dram_tensor("arg", [S, H], dt.float32, kind="ExternalInput")
    w_d = nc.dram_tensor("w_all", [1, H * D], dt.float32r, kind="ExternalInput")
    bo_bc = nc.dram_tensor("bo_bc", [P, D], dt.float32, kind="ExternalInput")
    ones_r_d = nc.dram_tensor("ones_r", [1, P], dt.float32r, kind="ExternalInput")
    ones8_d = nc.dram_tensor("ones8", [P, 2, 32], FP8, kind="ExternalInput")
    out = nc.dram_tensor("out", [S, D], dt.float32, kind="ExternalOutput")
    if dbg:
        dbg_tT = nc.dram_tensor("dbg_tT", [P, KC, S], dt.float32, kind="ExternalOutput")
        dbg_att8 = nc.dram_tensor("dbg_att8", [P, TC, D], dt.float32, kind="ExternalOutput")
        dbg_dn = nc.dram_tensor("dbg_dn", [P, KC], dt.float32, kind="ExternalOutput")
        dbg_rcol = nc.dram_tensor("dbg_rcol", [P, KC], dt.float32, kind="ExternalOutput")
        dbg_cx8 = nc.dram_tensor("dbg_cx8", [P, KC, D], dt.float32, kind="ExternalOutput")
        dbg_y0 = nc.dram_tensor("dbg_y0", [P, TC, D], dt.float32, kind="ExternalOutput")

    with tile.TileContext(nc) as tc, ExitStack() as ctx:
        pp = ctx.enter_context(tc.tile_pool(name="persist", bufs=1))
        inp = ctx.enter_context(tc.tile_pool(name="inp", bufs=2))
        wh = ctx.enter_context(tc.tile_pool(name="wh", bufs=wh_bufs))
        es_p = ctx.enter_context(tc.tile_pool(name="es", bufs=es_bufs))
        as_p = ctx.enter_context(tc.tile_pool(name="atts", bufs=es_bufs))
        att_p = ctx.enter_context(tc.tile_pool(name="att8", bufs=att_bufs))
        cx8_p = ctx.enter_context(tc.tile_pool(name="cx8", bufs=2))
        sm = ctx.enter_context(tc.tile_pool(name="small", bufs=3))
        dbg_p = ctx.enter_context(tc.tile_pool(name="dbgp", bufs=1))
        pe_ps = ctx.enter_context(tc.tile_pool(name="pe_ps", bufs=pe_bufs, space="PSUM"))
        sc_ps = ctx.enter_context(tc.tile_pool(name="sc_ps", bufs=sc_bufs, space="PSUM"))
        cx_ps = ctx.enter_context(tc.tile_pool(name="cx_ps", bufs=cx_bufs, space="PSUM"))

        def body(_iv=None):
            # ---- persistent loads ----
            eb_sb = inp.tile([P, TC, S], dt.bfloat16, tag="eb")
            nc.sync.dma_start(eb_sb[:], eb_d.rearrange("(c p) q -> p c q", p=P))
            xT_sb = inp.tile([P, KC, S], FP8, tag="xT")
            nc.sync.dma_start(xT_sb[:], xT.rearrange("(c p) n -> p c n", p=P))
            xN_sb = inp.tile([P, TC, D], FP8, tag="xN")
            nc.sync.dma_start(xN_sb[:], xN.rearrange("(c p) n -> p c n", p=P))
            arg_sb = inp.tile([P, TC, H], dt.float32, tag="arg")
            nc.sync.dma_start(arg_sb[:], arg_d.rearrange("(c p) h -> p c h", p=P))
            w_sb = pp.tile([1, H, D], dt.float32r, tag="w")
            nc.sync.dma_start(w_sb[:], w_d.rearrange("a (h n) -> a h n", h=H))
            bo_sb = pp.tile([P, D], dt.float32, tag="bo")
            nc.sync.dma_start(bo_sb[:], bo_bc[:, :])
            ones_r = pp.tile([1, P], dt.float32r, tag="ones_r")
            nc.sync.dma_start(ones_r[:], ones_r_d[:, :])
            ones8 = pp.tile([P, 2, 32], FP8, tag="ones8")
            nc.sync.dma_start(ones8[:], ones8_d[:, :, :])

            y = pp.tile([P, TC, D], dt.float32, tag="y")

            # ---- t computation, software-pipelined one head ahead ----
            tTs = {}

            def t_block(h):
                bh = wh.tile([P, KC, D], FP8, tag="bh")
                nc.sync.dma_start(bh[:], b_all.rearrange("(c p) n -> p c n", p=P)[:, :, h * D:(h + 1) * D])
                tT = pp.tile([P, KC, S], FP8, tag=f"tT{h}")
                tTs[h] = tT
                for mc in range(KC):
                    for nt in range(NT):
                        ps = pe_ps.tile([P, D], dt.float32, tag="pe")
                        for k2 in range(KC // 2):
                            nc.tensor.matmul(ps[:], bh[:, 2 * k2:2 * k2 + 2, mc * P:(mc + 1) * P],
                                             xT_sb[:, 2 * k2:2 * k2 + 2, nt * D:(nt + 1) * D],
                                             start=(k2 == 0), stop=(k2 == KC // 2 - 1),
                                             perf_mode=DR)
                        with nc.allow_low_precision(reason="fp8 tT, validated numerics"):
                            if (mc * NT + nt) % 2 == 0:
                                nc.scalar.copy(tT[:, mc, nt * D:(nt + 1) * D], ps[:])
                            else:
                                nc.vector.tensor_copy(tT[:, mc, nt * D:(nt + 1) * D], ps[:])

            t_block(0)

            # ---- head loop: ctx/proj delayed one head; ctx matmul chains
            # interleaved into the next head's att emission so the in-order
            # PE queue never waits on Act/DVE evictions ----
            state = {}

            def emit_ctx_dc(hp, dc):
                st = state[hp]
                pss = [None, None]
                for kp in range(TC // 2):
                    for qt in range(2):
                        if kp == 0:
                            ps_cx = cx_ps.tile([P, D], dt.float32, tag="cx")
                            pss[qt] = ps_cx
                        nc.tensor.matmul(pss[qt][:],
                                         xN_sb[:, 2 * kp:2 * kp + 2, dc * P:(dc + 1) * P],
                                         st["att8s"][qt][:, 2 * kp:2 * kp + 2, :],
                                         start=(kp == 0), stop=(kp == TC // 2 - 1),
                                         perf_mode=DR)
                for qt in range(2):
                    with nc.allow_low_precision(reason="fp8 ctx, validated numerics"):
                        nc.vector.tensor_scalar(st["cx8s"][qt][:, dc, :], pss[qt][:],
                                                S_C, None, ALU.mult)

            def att_phase(h, prev):
                st = state[h]
                a0 = att_p.tile([P, TC, D], FP8, tag="att8")
                a1 = att_p.tile([P, TC, D], FP8, tag="att8")
                st["att8s"] = [a0, a1]
                if prev is not None:
                    cx8_0 = cx8_p.tile([P, KC, D], FP8, tag="cx8")
                    cx8_1 = cx8_p.tile([P, KC, D], FP8, tag="cx8")
                    state[prev]["cx8s"] = [cx8_0, cx8_1]
                tT = tTs[h]
                for kt in range(TC):
                    for qt in range(2):
                        es = es_p.tile([P, D], dt.bfloat16, tag="es")
                        ps = sc_ps.tile([P, D], dt.float32, tag="sc")
                        for d2 in range(KC // 2):
                            nc.tensor.matmul(ps[:], xT_sb[:, 2 * d2:2 * d2 + 2, kt * P:(kt + 1) * P],
                                             tT[:, 2 * d2:2 * d2 + 2, qt * D:(qt + 1) * D],
                                             start=(d2 == 0), stop=(d2 == KC // 2 - 1),
                                             perf_mode=DR)
                        nc.scalar.activation(es[:], ps[:], AF.Exp,
                                             bias=arg_sb[:, kt, h:h + 1],
                                             scale=float(1.0 / KS))
                        ats = as_p.tile([P, D], dt.bfloat16, tag="ats")
                        nc.vector.tensor_tensor(ats[:], es[:],
                                                eb_sb[:, kt, qt * D:(qt + 1) * D],
                                                ALU.mult)
                        att8 = st["att8s"][qt]
                        with nc.allow_low_precision(reason="fp8 centered att, validated numerics"):
                            if (kt + qt) % 2 == 0:
                                nc.scalar.activation(att8[:, kt, :], ats[:], AF.Copy,
                                                     bias=-S_SC)
                            else:
                                nc.vector.tensor_scalar(att8[:, kt, :], ats[:],
                                                        S_SC, None, ALU.subtract)
                    if kt % 2 == 1 and prev is not None:
                        emit_ctx_dc(prev, kt // 2)

            def den_phase(h, qt):
                st = state[h]
                att8 = st["att8s"][qt]
                dps = cx_ps.tile([P, D], dt.float32, tag="cx")
                for kp in range(TC // 2):
                    nc.tensor.matmul(dps[0:32, :], ones8[:, :, :],
                                     att8[:, 2 * kp:2 * kp + 2, :],
                                     start=(kp == 0), stop=(kp == TC // 2 - 1),
                                     perf_mode=DR)
                dcp = sm.tile([1, D], dt.float32, tag="dcp")
                nc.vector.tensor_scalar(dcp[:], dps[0:1, :], S_SC * S, S_U * S_C,
                                        ALU.add, ALU.mult)
                dn_sb = sm.tile([P, KC], dt.float32, tag="dn_sb")
                for c in range(KC):
                    nc.gpsimd.dma_start(dn_sb[:, c:c + 1], dcp[0:1, c * P:(c + 1) * P])
                rcol = sm.tile([P, KC], dt.float32, tag="rcol")
                nc.vector.reciprocal(rcol[:], dn_sb[:])
                st.setdefault("rcols", {})[qt] = rcol

            def proj_phase(hp, qt):
                st = state[hp]
                cx8 = st["cx8s"][qt]
                rcol = st["rcols"][qt]
                u8h = st["u8h"]
                for qc in range(KC):
                    ps = pe_ps.tile([P, D], dt.float32, tag="pe")
                    nc.tensor.matmul(ps[:], ones_r[:], w_sb[0:1, hp, :],
                                     start=True, stop=False)
                    for c2 in range(KC // 2):
                        nc.tensor.matmul(ps[:], cx8[:, 2 * c2:2 * c2 + 2, qc * P:(qc + 1) * P],
                                         u8h[:, 2 * c2:2 * c2 + 2, :],
                                         start=False, stop=(c2 == KC // 2 - 1),
                                         perf_mode=DR)
                    ys = y[:, qt * KC + qc, :]
                    if hp == 0:
                        nc.vector.scalar_tensor_tensor(ys, ps[:], rcol[:, qc:qc + 1],
                                                       bo_sb[:], ALU.mult, ALU.add)
                    else:
                        nc.vector.scalar_tensor_tensor(ys, ps[:], rcol[:, qc:qc + 1],
                                                       ys, ALU.mult, ALU.add)

            for h in range(H):
                state[h] = {}
                if h + 1 < H:
                    t_block(h + 1)
                u8h = wh.tile([P, KC, D], FP8, tag="u8h")
                nc.sync.dma_start(u8h[:], u8_d.rearrange("(c p) n -> p c n", p=P)[:, :, h * D:(h + 1) * D])
                state[h]["u8h"] = u8h
                att_phase(h, h - 1 if h > 0 else None)
                den_phase(h, 0)
                den_phase(h, 1)
                if h > 0:
                    proj_phase(h - 1, 0)
                    proj_phase(h - 1, 1)
                    del state[h - 1]
            # flush last head
            hl = H - 1
            cx8_0 = cx8_p.tile([P, KC, D], FP8, tag="cx8")
            cx8_1 = cx8_p.tile([P, KC, D], FP8, tag="cx8")
            state[hl]["cx8s"] = [cx8_0, cx8_1]
            for dc in range(KC):
                emit_ctx_dc(hl, dc)
            proj_phase(hl, 0)
            proj_phase(hl, 1)

            # ---- out = y ----
            for tc_ in range(TC):
                nc.sync.dma_start(out[tc_ * P:(tc_ + 1) * P, :], y[:, tc_, :])

        if n_iters == 1:
            body()
        else:
            with tc.For_i(0, n_iters, 1) as iv:
                body(iv)

    nc.compile()
    return nc


def make_in_maps(inputs: dict, md_name: str = "fp8v2") -> list[dict]:
    import ml_dtypes
    f32 = np.float32
    bf16 = ml_dtypes.bfloat16
    fp8 = ml_dtypes.float8_e4m3
    x = np.ascontiguousarray(np.asarray(inputs["x"], f32))
    bias = np.asarray(inputs["attn_bias"], f32)[0, 0]
    Wq_lin = np.asarray(inputs["Wq_lin"], f32)
    bq = np.asarray(inputs["bq_lin"], f32)
    W_qk = np.asarray(inputs["W_qk"], f32)
    b_qk = np.asarray(inputs["b_qk"], f32)
    W_v = np.asarray(inputs["W_v"], f32)
    b_v = np.asarray(inputs["b_v"], f32)
    W_proj = np.asarray(inputs["W_proj"], f32)
    b_proj = np.asarray(inputs["b_proj"], f32)
    W_out = np.asarray(inputs["W_out"], f32)
    b_out = np.asarray(inputs["b_out"], f32)

    Wq, Wk = W_qk[:, :H * D], W_qk[:, H * D:]
    B_all = np.empty((D, H * D), f32)
    Wr = np.empty((D, H), f32)
    for h in range(H):
        Aq = Wq_lin @ Wq[:, h * D:(h + 1) * D]
        Ak = Wq_lin @ Wk[:, h * D:(h + 1) * D]
        aq = Wq[:, h * D:(h + 1) * D].T @ bq + b_qk[h * D:(h + 1) * D]
        B_all[:, h * D:(h + 1) * D] = SQ * (Aq @ Ak.T)
        Wr[:, h] = Ak @ aq
    Wv2 = Wq_lin @ W_v
    bv2 = W_v.T @ bq + b_v
    Wo2 = W_proj @ W_out
    bo2 = W_out.T @ (W_proj.T @ bv2 + b_proj) + b_out
    Uh = np.empty((D, H * D), f32)
    for h in range(H):
        Uh[:, h * D:(h + 1) * D] = Wv2[:, h * D:(h + 1) * D] @ Wo2[h * D:(h + 1) * D, :]

    shared = {
        "b_all": np.ascontiguousarray(B_all * KS).astype(fp8),
        "u8": np.ascontiguousarray(Uh * S_U).astype(fp8),
        "eb": np.ascontiguousarray(np.exp(bias.T * SQ)).astype(bf16),
        "bo_bc": np.broadcast_to(bo2, (P, D)).copy(),
        "ones_r": np.ones((1, P), f32),
        "ones8": np.ones((P, 2, 32), fp8),
    }
    maps = []
    for b in range(8):
        xb = x[b]
        arg = (xb @ Wr) * SQ + np.log(S_SC)                  # [S, H] f32
        colsumx = xb.sum(0)                                  # [D]
        w_all = np.empty((H * D,), f32)
        for h in range(H):
            w_all[h * D:(h + 1) * D] = (Uh[:, h * D:(h + 1) * D].T @ colsumx) * (S_SC * S_U * S_C)
        maps.append({
            "xT": np.ascontiguousarray(xb.T).astype(fp8),
            "xN": np.ascontiguousarray(xb).astype(fp8),
            "arg": np.ascontiguousarray(arg),
            "w_all": w_all[None, :].astype(f32),
            **shared,
        })
    return maps


MD_NAME = "fp8v2"
CFG = dict(sc_bufs=4, cx_bufs=2, pe_bufs=2, att_bufs=4, es_bufs=6, wh_bufs=2, quant_pool_qt=2)

_BUILT = {}


def _get_nc():
    if "nc" not in _BUILT:
        _BUILT["nc"] = build(n_iters=1, md_name=MD_NAME, **CFG)
    return _BUILT["nc"]


def kernel(**inputs) -> np.ndarray:
    from concourse.bass_utils import run_bass_kernel_spmd

    nc = _get_nc()
    in_maps = make_in_maps(inputs, MD_NAME)
    r = run_bass_kernel_spmd(nc, in_maps, core_ids=list(range(8)))
    out = np.stack([r.results[b]["out"] for b in range(8)]).astype(np.float32)
    return out
